# revision 10
# baseline (speedup 1.0000x reference)
"""Multi-head attention (b=4, c=256, l=2048, 8 heads x 64) on 8 TRN2 NeuronCores.

Sharding: core i handles batch b = i//2 and query half qh = i%2 (1024 queries),
computing all 8 heads over the full 2048-key context. Outputs are disjoint
[256, 1024] slabs -> host-side concat only, no collectives. The core's query
half is ROLLED to the front of its x copy host-side (keys are permuted
consistently for K and V, softmax is permutation-invariant), so no separate
xq input is sent.

v2 redesign on top of the measured 175.2us baseline:
 - PE is the real floor (~130us of matmul free-dim cycles: QK 54.6 + PV 54.6
   + qkv-proj 17.1 + out-proj 3.4us). ACT was the pacer (140.6us busy) only
   because it carried ALL 16.78M exps. A fraction of exp chunks now runs on
   DVE as a one-instruction Schraudolph: i16(a*s + b) bitcast as bf16
   (exp2 via exponent-field arithmetic; 2.06% rms sawtooth measured on HW;
   fraction phi~0.23 -> ~1.1e-2 total rel err vs the 2e-2 budget). GPSIMD
   cannot access PSUM (BIR verifier rejects), so scores can only be exp'd by
   ACT or DVE.
 - Seg order is query-half-major: (p,ih) = (0,0),(1,0),(2,0),(3,0),(0,1)...
   so attn[:, 0:512] completes 4 segs early; the out-projection for those
   columns accumulates per-p into a 2-bank PSUM pool and its output DMA
   issues mid-stream, halving the end-of-kernel DMA drain.
 - Uniform [128,1024] score chunks (1 key-tile x 2 heads); psum pairs:
   pv (banks 0-1), qkA (2-3), qkB (4-5), proj->oproj (6-7): ACT/DVE chunk
   reads never share a bank pair with concurrent PE writes.
 - proj tasks spread over the 4 ih=0 segs (PE smoothing); 14 warmup dummy
   matmuls fill the DMA lead-in so the PE p-state is hot when the real
   projection chain lands.

Hard-won HW facts kept from the baseline (re-verified by microbench):
 1. CLOCK POISON: InstReciprocal, gpsimd partition_broadcast, or
    scalar.activation(Identity, bias=AP) anywhere drops EVERY engine's clock
    1.2x. Normalization stays on the poison-free DVE chain:
    reciprocal_approx_fast + STREAM_SHUFFLE broadcast + copy + TT.
    (gpsimd TensorScalar/TensorTensor software ops do NOT poison: ACT exp
    stayed at 1967ns with them present.)
 2. ACT cost = (N + ~310)/1.2GHz per ACTIVATE. DVE tensor_scalar from PSUM =
    (N + ~120)/0.96GHz; no 16-bit speedup possible (fp32 input).
 3. DMA engines take ~5us from first descriptor to first packet; critical
    slices (pair-0 weights, first x columns) are issued first.
"""

import sys

if "/opt/trn_rl_repo" not in sys.path:
    sys.path.insert(0, "/opt/trn_rl_repo")

import numpy as np

import concourse.bass as bass
import concourse.mybir as mybir
import concourse.tile as tile
from concourse import bacc
from concourse.bass_utils import run_bass_kernel_spmd

F32 = mybir.dt.float32
BF16 = mybir.dt.bfloat16
I16 = mybir.dt.int16
EXP = mybir.ActivationFunctionType.Exp
MULT = mybir.AluOpType.mult
ADD = mybir.AluOpType.add

B, C, L = 4, 256, 2048
H, D = 8, 64
HID = H * D  # 512
LQ = L // 2  # 1024 queries per core
NJT = L // 128  # 16 key tiles == chunks per seg
SCALE = D**-0.5

# Schraudolph exp-as-bf16-bits: bf16_bits(i16(A*s + B)) ~= exp(SCALE*s)
SCH_A = SCALE * 1.4426950408889634 * 128.0
SCH_B = 16256.0 - 5.513  # 127*128 minus the balanced-sawtooth shift

# chunks (of 16 per seg) computed on DVE instead of ACT, per seg index.
# early chunks of each seg are avoided (DVE runs the previous seg's norm
# there); seg 0 keeps DVE mostly free for the projection casts.
DVE_CHUNKS = {
    0: (7, 11),
    1: (5, 8, 11, 14),
    2: (5, 8, 11, 14),
    3: (5, 8, 11, 14),
    4: (4, 7, 10, 13),
    5: (4, 7, 10, 13),
    6: (4, 7, 10, 13),
    7: (6, 9, 12),
}

SEG_ORDER = [(0, 0), (1, 0), (2, 0), (3, 0), (0, 1), (1, 1), (2, 1), (3, 1)]
SEG_LAG = [4, 4, 4, 4, 3, 3, 3, 3]

DEBUG_DUMP = True

_cached = {}


def build_nc():
    nc = bacc.Bacc(
        "TRN2",
        target_bir_lowering=False,
        debug=False,
        enable_asserts=False,
        num_devices=8,
    )
    x_d = nc.dram_tensor("x", [C, L], BF16, kind="ExternalInput")
    wq_d = nc.dram_tensor("wqkvT", [C, 3 * HID], BF16, kind="ExternalInput")
    wo_d = nc.dram_tensor("woutT", [HID, C], BF16, kind="ExternalInput")
    bias_d = nc.dram_tensor("bias", [C, 1], F32, kind="ExternalInput")
    out_d = nc.dram_tensor("out", [C, LQ], BF16, kind="ExternalOutput")
    if DEBUG_DUMP:
        attn_d = nc.dram_tensor("attn_dbg", [4, 128, LQ], BF16, kind="ExternalOutput")
        q_dbg = nc.dram_tensor("q_dbg", [4, 128, LQ], BF16, kind="ExternalOutput")
        k_dbg = nc.dram_tensor("k_dbg", [4, 128, L], BF16, kind="ExternalOutput")
        vt_dbg = nc.dram_tensor("vt_dbg", [4, 128, H * (D + 1)], BF16, kind="ExternalOutput")

    with tile.TileContext(nc) as tc:
        with (
            tc.tile_pool(name="const", bufs=1) as cp,
            tc.tile_pool(name="epool", bufs=10) as ep,
            tc.tile_pool(name="opool", bufs=2) as op,
            tc.tile_pool(name="pvps", bufs=1, space=bass.MemorySpace.PSUM) as pvps,
        ):
            # ---- persistent SBUF tensors ----
            xb = [cp.tile([128, L], BF16, tag=f"xb{k}", name=f"xb{k}") for k in range(2)]
            wq = [cp.tile([128, 3 * HID], BF16, tag=f"wq{k}", name=f"wq{k}") for k in range(2)]
            wo = [cp.tile([128, C], BF16, tag=f"wo{k}", name=f"wo{k}") for k in range(4)]
            bias = [cp.tile([128, 1], F32, tag=f"bias{k}", name=f"bias{k}") for k in range(2)]
            Qs = [cp.tile([128, LQ], BF16, tag=f"Q{m}", name=f"Q{m}") for m in range(4)]
            Ks = [cp.tile([128, L], BF16, tag=f"K{m}", name=f"K{m}") for m in range(4)]
            VT = [cp.tile([128, H, D + 1], BF16, tag=f"VT{t}", name=f"VT{t}") for t in range(NJT)]
            attn = [cp.tile([128, LQ], BF16, tag=f"attn{m}", name=f"attn{m}") for m in range(4)]
            pons = [cp.tile([D + 1, 512], F32, tag=f"pons{k}", name=f"pons{k}") for k in range(4)]
            dens = [cp.tile([1, 512], F32, tag=f"den{k}", name=f"den{k}") for k in range(4)]
            recs = [cp.tile([32, 512], F32, tag=f"rec{k}", name=f"rec{k}") for k in range(2)]
            rbcs = [cp.tile([64, 512], F32, tag=f"rbc{k}", name=f"rbc{k}") for k in range(4)]
            dum = cp.tile([1, 16], F32, tag="dum", name="dum")
            dumo = cp.tile([1, 16], F32, tag="dumo", name="dumo")
            wdum = cp.tile([128, 128], BF16, tag="wdum", name="wdum")
            rdum = cp.tile([128, 512], BF16, tag="rdum", name="rdum")

            # warmups: exp table load on ACT; PE clock-ramp dummies come after
            # the psum pools open -- see below.
            nc.vector.memset(dum[:], 1.0)
            nc.vector.memset(wdum[:], 0.125)
            nc.vector.memset(rdum[:], 0.125)
            nc.scalar.activation(dumo[:], dum[:], EXP)
            for k in range(2):
                nc.vector.memset(recs[k][:], 0.0)

            # ---- DMA: critical slices interleaved across sync+gpsimd queues
            nc.sync.dma_start(wq[0][:, 0:128], wq_d.ap()[0:128, 0:128])
            nc.gpsimd.dma_start(wq[1][:, 0:128], wq_d.ap()[128:256, 0:128])
            nc.sync.dma_start(wq[0][:, 512:640], wq_d.ap()[0:128, 512:640])
            nc.gpsimd.dma_start(wq[1][:, 512:640], wq_d.ap()[128:256, 512:640])
            nc.sync.dma_start(xb[0][:, 0:512], x_d.ap()[0:128, 0:512])
            nc.gpsimd.dma_start(xb[1][:, 0:512], x_d.ap()[128:256, 0:512])
            nc.sync.dma_start(wq[0][:, 1024:1536], wq_d.ap()[0:128, 1024:1536])
            nc.gpsimd.dma_start(wq[1][:, 1024:1536], wq_d.ap()[128:256, 1024:1536])
            nc.sync.dma_start(xb[0][:, 512:1024], x_d.ap()[0:128, 512:1024])
            nc.gpsimd.dma_start(xb[1][:, 512:1024], x_d.ap()[128:256, 512:1024])
            # VT ones columns (no deps)
            for t in range(NJT):
                nc.gpsimd.memset(VT[t][:, :, D : D + 1], 1.0)
            # bulk
            nc.sync.dma_start(xb[0][:, 1024:2048], x_d.ap()[0:128, 1024:2048])
            nc.gpsimd.dma_start(xb[1][:, 1024:2048], x_d.ap()[128:256, 1024:2048])
            nc.sync.dma_start(wq[0][:, 128:512], wq_d.ap()[0:128, 128:512])
            nc.gpsimd.dma_start(wq[1][:, 128:512], wq_d.ap()[128:256, 128:512])
            nc.sync.dma_start(wq[0][:, 640:1024], wq_d.ap()[0:128, 640:1024])
            nc.gpsimd.dma_start(wq[1][:, 640:1024], wq_d.ap()[128:256, 640:1024])
            for k in range(4):
                q = nc.sync if k % 2 == 0 else nc.gpsimd
                q.dma_start(wo[k][:], wo_d.ap()[128 * k : 128 * (k + 1), :])
            for k in range(2):
                rows = slice(128 * k, 128 * (k + 1))
                q = nc.sync if k % 2 == 0 else nc.gpsimd
                q.dma_start(bias[k][:], bias_d.ap()[rows, :])

            # ---- projection task machinery ----
            task_pool = [None]  # set once proj psum pool opens

            def q_task(p, n):
                ps = task_pool[0].tile([128, 512], F32, tag="proj", name="psq")
                for k in range(2):
                    nc.tensor.matmul(
                        ps[:],
                        wq[k][:, 128 * p : 128 * (p + 1)],
                        xb[k][:, 512 * n : 512 * (n + 1)],
                        start=(k == 0),
                        stop=(k == 1),
                    )
                nc.vector.tensor_copy(Qs[p][:, 512 * n : 512 * (n + 1)], ps[:])

            def k_task(p, j):
                ps = task_pool[0].tile([128, 512], F32, tag="proj", name="psk")
                for k in range(2):
                    nc.tensor.matmul(
                        ps[:],
                        wq[k][:, HID + 128 * p : HID + 128 * (p + 1)],
                        xb[k][:, 512 * j : 512 * (j + 1)],
                        start=(k == 0),
                        stop=(k == 1),
                    )
                nc.vector.tensor_copy(Ks[p][:, 512 * j : 512 * (j + 1)], ps[:])

            def vt_task(t):
                ps = task_pool[0].tile([128, 512], F32, tag="proj", name="psv")
                for k in range(2):
                    nc.tensor.matmul(
                        ps[:],
                        xb[k][:, 128 * t : 128 * (t + 1)],
                        wq[k][:, 2 * HID : 3 * HID],
                        start=(k == 0),
                        stop=(k == 1),
                    )
                nc.vector.tensor_copy(
                    VT[t][:, :, 0:D], ps[:].rearrange("p (h c) -> p h c", h=H)
                )

            # Per-seg {chunk: [task]} schedules, tuned so each K/Q block and
            # VT tile lands ~2 chunks before its first consumer (QK reads K at
            # chunk 4j; the lag-7 PV flush reads VT[c] at global chunk c+7).
            TASKS = {
                0: {0: [lambda: k_task(0, 1)], 1: [lambda: vt_task(4)],
                    2: [lambda: vt_task(5)], 3: [lambda: k_task(0, 2)],
                    4: [lambda: vt_task(6)], 5: [lambda: vt_task(7)],
                    6: [lambda: k_task(0, 3)], 7: [lambda: vt_task(8)],
                    8: [lambda: q_task(1, 0)], 9: [lambda: k_task(1, 0)],
                    10: [lambda: vt_task(9)], 11: [lambda: vt_task(10)],
                    12: [lambda: vt_task(11)], 13: [lambda: vt_task(12)],
                    14: [lambda: vt_task(13)], 15: [lambda: vt_task(14)]},
                1: {0: [lambda: vt_task(15)], 1: [lambda: k_task(1, 1)],
                    2: [lambda: k_task(1, 2)], 3: [lambda: k_task(1, 3)],
                    4: [lambda: k_task(2, 0)], 5: [lambda: q_task(2, 0)]},
                2: {0: [lambda: k_task(2, 1)], 1: [lambda: k_task(2, 2)],
                    2: [lambda: k_task(2, 3)], 3: [lambda: k_task(3, 0)],
                    4: [lambda: q_task(3, 0)], 5: [lambda: q_task(0, 1)]},
                3: {0: [lambda: k_task(3, 1)], 1: [lambda: k_task(3, 2)],
                    2: [lambda: k_task(3, 3)], 3: [lambda: q_task(1, 1)],
                    4: [lambda: q_task(2, 1)], 5: [lambda: q_task(3, 1)]},
            }

            # ---- normalization chain (poison-free: no InstReciprocal, no
            # partition_broadcast). custom-DVE ops misread partition-offset
            # inputs, so the den row (partition 64) is staged to offset 0.
            norm_i = [0]

            def do_norm(src_pair, p, ih):
                cols = slice(512 * ih, 512 * (ih + 1))
                for s in (0, 1):
                    i = norm_i[0]
                    norm_i[0] += 1
                    rec = recs[i % 2]
                    rbc = rbcs[i % 4]
                    den = dens[i % 4]
                    nc.vector.tensor_copy(den[:], src_pair[s][D : D + 1, :])
                    nc.vector.reciprocal_approx_fast(rec[0:1, :], den[:])
                    nc.vector.stream_shuffle(rbc[0:32, :], rec[0:32, :], [0] * 32)
                    nc.vector.tensor_copy(rbc[32:64, :], rbc[0:32, :])
                    nc.vector.tensor_tensor(
                        attn[p][64 * s : 64 * (s + 1), cols],
                        src_pair[s][0:D, :],
                        rbc[:],
                        MULT,
                    )

            def pv_flush(E, c, po, p):
                for s in (0, 1):
                    nc.tensor.matmul(
                        po[s][:],
                        VT[c][:, 2 * p + s, :],
                        E[:, 512 * s : 512 * (s + 1)],
                        start=(c == 0),
                        stop=(c == NJT - 1),
                    )

            # ---- out-projection accumulation ----
            oproj_pool = [None]
            oproj_ps = [None, None]  # per m

            def oproj_open(ih):
                for m in range(2):
                    oproj_ps[m] = oproj_pool[0].tile(
                        [128, 512], F32, tag=f"om{m}", name=f"om{m}_{ih}"
                    )

            def oproj_add(p, ih, split=False):
                cols = slice(512 * ih, 512 * (ih + 1))
                for m in range(2):
                    if split:
                        # split p3 by head-half so the s=0 half can issue
                        # while the s=1 norm chain still runs (tail path)
                        for half in range(2):
                            hr = slice(64 * half, 64 * (half + 1))
                            nc.tensor.matmul(
                                oproj_ps[m][:],
                                wo[p][hr, 128 * m : 128 * (m + 1)],
                                attn[p][hr, cols],
                                start=False,
                                stop=(half == 1),
                            )
                    else:
                        nc.tensor.matmul(
                            oproj_ps[m][:],
                            wo[p][:, 128 * m : 128 * (m + 1)],
                            attn[p][:, cols],
                            start=(p == 0),
                            stop=(p == 3),
                        )

            def oproj_finish(ih):
                for m in range(2):
                    # unique tag per (ih, m): osb's only reader is the output
                    # DMA -- never reuse a DMA-read tile within a run.
                    osb = op.tile([128, 512], BF16, tag=f"osb{ih}{m}", name=f"osb{m}_{ih}")
                    # NOTE: scalar.activation(Identity, bias=AP) triggers the
                    # global 1.2x clock slowdown -- keep bias adds on DVE.
                    nc.vector.tensor_scalar_add(osb[:], oproj_ps[m][:], bias[m][:])
                    deng = [nc.sync, nc.gpsimd, nc.scalar, nc.sync][2 * ih + m]
                    deng.dma_start(
                        out_d.ap()[128 * m : 128 * (m + 1), 512 * ih : 512 * (ih + 1)],
                        osb[:],
                    )
                    oproj_ps[m] = None

            # ---- the stream ----
            pending = []  # FIFO of (E, c, po, p, ih, seg)

            def seg_finish(po, p, ih, seg):
                if seg == 7:
                    # nothing follows: normalize straight from PV psum, no
                    # psum-freeing copies on the critical tail path.
                    do_norm(po, p, ih)
                else:
                    pp = [pons[2 * (seg % 2) + s] for s in (0, 1)]
                    for s in (0, 1):
                        nc.vector.tensor_copy(pp[s][:], po[s][:])
                    do_norm(pp, p, ih)
                pass

            def run_seg(seg, qk_pools, extras=()):
                p, ih = SEG_ORDER[seg]
                lag = SEG_LAG[seg]
                sched = TASKS.get(seg, {})
                extras = dict(extras)
                Qh = [
                    Qs[p][64 * s : 64 * (s + 1), 512 * ih : 512 * (ih + 1)]
                    for s in (0, 1)
                ]
                Kh = [Ks[p][64 * s : 64 * (s + 1), :] for s in (0, 1)]
                po = [
                    pvps.tile([D + 1, 512], F32, tag=f"pv{s}", name=f"po{s}")
                    for s in (0, 1)
                ]
                dve_set = DVE_CHUNKS.get(seg, ())
                for c in range(NJT):
                    qkps = qk_pools[c % 2]
                    ps = qkps.tile([128, 1024], F32, tag=f"qk{c % 2}", name="psqk")
                    for s in (0, 1):
                        nc.tensor.matmul(
                            ps[:, 512 * s : 512 * (s + 1)],
                            Kh[s][:, 128 * c : 128 * (c + 1)],
                            Qh[s][:],
                            start=True,
                            stop=True,
                        )
                    pops = 0
                    while pending and len(pending) > lag and pops < 2:
                        ent = pending.pop(0)
                        pv_flush(*ent[:4])
                        if ent[1] == NJT - 1:
                            seg_finish(ent[2], ent[3], ent[4], ent[5])
                        pops += 1
                    E = ep.tile([128, 1024], BF16, tag="e", name="E")
                    if c in dve_set:
                        nc.vector.tensor_scalar(
                            E[:].bitcast(I16), ps[:], SCH_A, SCH_B, MULT, ADD
                        )
                    else:
                        nc.scalar.activation(E[:], ps[:], EXP, scale=SCALE)
                    pending.append((E, c, po, p, ih, seg))
                    for fn in sched.get(c, ()):
                        fn()
                    if c in extras:
                        extras[c]()

            def drain_pending():
                while pending:
                    ent = pending.pop(0)
                    pv_flush(*ent[:4])
                    if ent[1] == NJT - 1:
                        seg_finish(ent[2], ent[3], ent[4], ent[5])

            with (
                tc.tile_pool(name="qkA", bufs=1, space=bass.MemorySpace.PSUM) as qkA,
                tc.tile_pool(name="qkB", bufs=1, space=bass.MemorySpace.PSUM) as qkB,
            ):
                pools = (qkA, qkB)
                with tc.tile_pool(name="projps", bufs=2, space=bass.MemorySpace.PSUM) as projp:
                    task_pool[0] = projp
                    # PE clock-ramp warmups during the DMA lead-in (no input
                    # deps): keep the p-state hot until the first real matmul.
                    for i in range(14):
                        ps = qkA.tile([128, 1024], F32, tag="qk0", name="warm")
                        nc.tensor.matmul(ps[:, 0:512], wdum[:], rdum[:],
                                         start=True, stop=True)
                    # pre-phase: first Q/K blocks + the first four VT tiles
                    # (baseline-stable shape: everything the first chunks and
                    # early PV flushes touch is produced before the stream)
                    q_task(0, 0)
                    k_task(0, 0)
                    vt_task(0)
                    vt_task(1)
                    vt_task(2)
                    vt_task(3)
                    # ih=0 segs carry the projection tasks
                    for seg in range(4):
                        run_seg(seg, pools)
                with tc.tile_pool(name="ops", bufs=1, space=bass.MemorySpace.PSUM) as opsp:
                    oproj_pool[0] = opsp
                    # cols 0:512 out-projection: all four attn col-halves are
                    # normalized; accumulate + DMA while ih=1 streams. Spread
                    # the matmul burst across seg 4's chunks.
                    for seg in range(4, 8):
                        run_seg(seg, pools)
                    drain_pending()
                    for ih in range(2):
                        oproj_open(ih)
                        for p in range(4):
                            oproj_add(p, ih)
                        oproj_finish(ih)
                    if DEBUG_DUMP:
                        for p in range(4):
                            nc.sync.dma_start(attn_d.ap()[p], attn[p][:])
                            nc.sync.dma_start(q_dbg.ap()[p], Qs[p][:])
                            nc.sync.dma_start(k_dbg.ap()[p], Ks[p][:])
                            nc.sync.dma_start(
                                vt_dbg.ap()[p],
                                VT[4 * p][:].rearrange("p h c -> p (h c)"),
                            )

    nc.compile()
    return nc


def get_nc():
    if "nc" not in _cached:
        _cached["nc"] = build_nc()
    return _cached["nc"]


def make_in_maps(x, w_qkv, w_out, b_out):
    import ml_dtypes

    bf16 = ml_dtypes.bfloat16
    wqkvT = np.ascontiguousarray(w_qkv.T.astype(bf16))
    woutT = np.ascontiguousarray(w_out.T.astype(bf16))
    bias = np.ascontiguousarray(b_out.astype(np.float32).reshape(C, 1))
    in_maps = []
    for i in range(8):
        b, qh = i // 2, i % 2
        xbf = x[b].astype(bf16)
        # roll keys so this core's query half occupies columns 0:LQ; K and V
        # see the same permutation so attention output is unchanged.
        xrot = np.ascontiguousarray(np.roll(xbf, -qh * LQ, axis=1))
        in_maps.append({"x": xrot, "wqkvT": wqkvT, "woutT": woutT, "bias": bias})
    return in_maps


def assemble(results):
    out = np.empty((B, C, L), dtype=np.float32)
    for i in range(8):
        b, qh = i // 2, i % 2
        out[b][:, qh * LQ : (qh + 1) * LQ] = np.asarray(
            results[i]["out"], dtype=np.float32
        )
    return out


def kernel(x, w_qkv, w_out, b_out):
    x = np.asarray(x, dtype=np.float32)
    w_qkv = np.asarray(w_qkv, dtype=np.float32)
    w_out = np.asarray(w_out, dtype=np.float32)
    b_out = np.asarray(b_out, dtype=np.float32)
    assert x.shape == (B, C, L), x.shape
    nc = get_nc()
    in_maps = make_in_maps(x, w_qkv, w_out, b_out)
    res = run_bass_kernel_spmd(nc, in_maps, list(range(8)), trace=False)
    return assemble(res.results)
